# revision 71
# baseline (speedup 1.0000x reference)
"""Trainium2 Bass kernel for a prenorm transformer Block (B=8, N=1024, D=768,
12 heads, MLP hidden 3072), data-parallel over batch across 8 NeuronCores.

Layout: activations transposed on-device (features on partitions, tokens on
the free dim) so the whole chain runs without on-device transposes.

Design highlights (v3):
  - QKV / attention-context / proj / fc1 matmuls run in fp8e4m3 DoubleRow
    (two 128-row contraction chunks per instruction, 2x PE throughput).
    All fp8 scales are exact powers of two picked on the host from the
    actual inputs; descales fold into existing PSUM-evacuation ops.
    fc2 stays fp16 (fp8 there costs ~1.3e-2 rel err on its own).
  - The attention window is Activation-engine bound (exp of all 12.6M
    scores).  QKV projection work for head-pair p+1 is software-pipelined
    INTO head-pair p's score/exp loop as PE filler, which both hides the
    QKV phase entirely and keeps the PE streaming (avoiding its
    half-clock pstate after stalls).  All QKV PSUM evacuations run on the
    DVE so the ACT engine does nothing but exp.
  - Softmax: scoresT layout, exp with a +kp*ln2 bias (fp8 output), ones
    column on V for denominators, context accumulators staged to SBUF
    immediately (frees PSUM for the next head pair), denominator rows
    batch-inverted with one DVE reciprocal per head pair, partition
    broadcast via DRAM roundtrip off the critical path.
  - fp16 residual stream; LayerNorm stats via PE ones-matmuls; the
    1/sqrt(var+eps) uses exp(-0.5*ln(var+eps)) on ACT (Ln and Exp share
    one table with the softmax exp, so no table reloads); per-token
    scale/shift rows broadcast on the PE into PSUM, affine is two
    2x-mode fp16 DVE passes.
  - fc2 accumulates over all 24 hidden chunks in PSUM; residual + bias
    via one fused scalar_tensor_tensor.
  - gamma/beta fold away when they are ones/zeros (checked on host at
    build time; a generic tensor_scalar pass is emitted otherwise).
"""
import sys
import types

sys.path.insert(0, "/opt/trn_rl_repo")

# concourse.bass_utils imports antenv.axon_hooks when tracing is requested;
# provide a no-op registry if the container image lacks that module so a
# BASS_TRACE=1 environment degrades to "no trace" instead of crashing.
try:
    import antenv.axon_hooks  # noqa: F401
except Exception:
    try:
        import antenv

        _hooks = types.ModuleType("antenv.axon_hooks")
        _hooks._hook = None

        def _set_hook(h):
            _hooks._hook = h

        def _get_hook():
            return _hooks._hook

        _hooks.set_axon_ntff_profile_hook = _set_hook
        _hooks.get_axon_ntff_profile_hook = _get_hook
        sys.modules["antenv.axon_hooks"] = _hooks
        antenv.axon_hooks = _hooks
    except Exception:
        pass

# boot() registers the NTFF profile hook only if antenv.axon_hooks exists at
# interpreter start; on this image it doesn't, so register it here through the
# shim so BASS_TRACE=1 yields exec times + perfetto traces.
try:
    import antenv.axon_hooks as _ah

    if _ah.get_axon_ntff_profile_hook() is None:
        from trn_agent_boot.trn_boot import _ntff_profile_via_ctypes

        _hk = _ntff_profile_via_ctypes("/opt/axon/libaxon_pjrt.so")
        if _hk is not None:
            _ah.set_axon_ntff_profile_hook(_hk)
except Exception:
    pass

import math

import ml_dtypes
import numpy as np

import concourse.bass as bass
import concourse.tile as tile
from concourse import mybir
from concourse.bass_utils import run_bass_kernel_spmd

F32 = mybir.dt.float32
F16 = mybir.dt.float16
F8 = mybir.dt.float8e4
AF = mybir.ActivationFunctionType
OP = mybir.AluOpType
DR = mybir.MatmulPerfMode.DoubleRow
NP_F8 = ml_dtypes.float8_e4m3  # TRN FP8_EXP4: max +-240

NCORES = 8
D, HEADS, HID, N = 768, 12, 3072, 1024
HD = D // HEADS                  # 64 head dim
DC = D // 128                    # 6 feature chunks
NB = N // 512                    # 2 moving-dim blocks
MT = N // 128                    # 8 key tiles
FCH = HID // 128                 # 24 hidden chunks
EPS = 1e-6
FC1_FP8 = True                   # fc1 in fp8 DoubleRow (fc2 stays fp16)

LAST_RESULT = None               # BassKernelResults of the most recent run


# The walrus build in this container rejects instructions carrying more than
# a couple of sync waits ("Too many sync wait commands"); fp8/fp16 matmuls
# reject more than one. Excess waits are hoisted onto standalone
# EventSemaphore carriers placed right before the instruction on the same
# engine, which is semantically identical (waits gate the engine stream).
_MM_OPS = ("Matmult", "Ldweights")


def _split_excess_waits(nc, default_limit=1, matmul_limit=0):
    counter = 0
    for f in nc.m.functions:
        for bb in f.blocks:
            new_insts = []
            for inst in bb.instructions:
                si = inst.sync_info
                waits = list(si.on_wait) if si and si.on_wait else []
                limit = matmul_limit if inst.opcode in _MM_OPS else default_limit
                if len(waits) > limit:
                    keep, move = waits[:limit], waits[limit:]
                    for w in move:
                        counter += 1
                        ev = mybir.InstEventSemaphore(
                            name=f"I-waitsplit-{counter}",
                            engine=inst.engine,
                            sync_info=mybir.SyncInfo(on_wait=[w], on_update=[]),
                        )
                        nc.register_instruction(ev, overwrite=True)
                        new_insts.append(ev)
                    inst.sync_info = mybir.SyncInfo(
                        on_wait=keep, on_update=list(si.on_update) if si else []
                    )
                new_insts.append(inst)
            bb.instructions = new_insts
    return counter


def _build(sc):
    """sc: dict of integer scale exponents + gamma/beta fast-path flags."""
    nc = bass.Bass()

    fc1_fp8 = sc["fc1_fp8"]
    xTb = nc.dram_tensor("xTb", [D, N], F32, kind="ExternalInput")
    xT8 = nc.dram_tensor("xT8", [D, N], F8, kind="ExternalInput")
    wqkvT8 = nc.dram_tensor("wqkvT8", [D, 3 * D], F8, kind="ExternalInput")
    wprojT8 = nc.dram_tensor("wprojT8", [D, D], F8, kind="ExternalInput")
    wfc1T = nc.dram_tensor("wfc1T", [D, HID], F8 if fc1_fp8 else F16,
                           kind="ExternalInput")
    wfc2T = nc.dram_tensor("wfc2T", [HID, D], F16, kind="ExternalInput")
    bfc1C = nc.dram_tensor("bfc1C", [128, FCH], F32, kind="ExternalInput")
    bfc2C = nc.dram_tensor("bfc2C", [128, DC], F32, kind="ExternalInput")
    gb1C = nc.dram_tensor("gb1C", [128, 2 * DC], F32, kind="ExternalInput")
    gb2C = nc.dram_tensor("gb2C", [128, 2 * DC], F32, kind="ExternalInput")
    yT = nc.dram_tensor("yT", [D, N], F32, kind="ExternalOutput")

    s_q = 2.0 ** (-sc["kx"] - sc["kq"])          # psum -> true q
    s_k = 2.0 ** (-sc["kx"] - sc["kk"])
    s_v = 2.0 ** (sc["kv"] - sc["kx"] - sc["kvw"])   # psum -> 2^kv * v
    s_ctx = 2.0 ** (sc["kc"] - sc["kv"])             # craw -> 2^kc * ctx
    s_pj = 2.0 ** (-sc["kc"] - sc["kpr"])            # psum -> true attn_out
    s_f1 = 2.0 ** (-sc["kx1"] - sc["kw1"]) if fc1_fp8 else 1.0
    exp_bias = float(sc["kp"] * math.log(2.0))       # exp(s + kp ln2)

    with tile.TileContext(nc) as tc:
        # ---------------- pools ----------------
        const = tc.alloc_tile_pool(name="const", bufs=1)
        p_w1 = tc.alloc_tile_pool(name="p_w1", bufs=1)
        stats = tc.alloc_tile_pool(name="stats", bufs=1)
        bc = tc.alloc_tile_pool(name="bc", bufs=2)
        p_xTb = tc.alloc_tile_pool(name="p_xTb", bufs=1, side="right")
        p_ctx = tc.alloc_tile_pool(name="p_ctx", bufs=1, side="right")
        p_attn = tc.alloc_tile_pool(name="p_attn", bufs=1, side="right")
        p_qkv_in = tc.alloc_tile_pool(name="p_qkv_in", bufs=1, side="right")
        p_craw = tc.alloc_tile_pool(name="p_craw", bufs=1, side="right")
        p_ae = tc.alloc_tile_pool(name="p_ae", bufs=2, side="right")
        dscr = tc.alloc_tile_pool(name="dscr", bufs=4, space="DRAM")

        ones_row16 = const.tile([1, 128], F16)
        nc.vector.tensor_copy(ones_row16[:], nc.const_aps.tensor(1.0, (1, 128)))
        ones16 = const.tile([128, 1], F16)
        nc.vector.tensor_copy(ones16[:], nc.const_aps.tensor(1.0, (128, 1)))
        expb_t = const.tile([128, 1], F32)
        nc.vector.memset(expb_t[:], exp_bias)
        eps_t = const.tile([1, 1], F32)
        nc.vector.memset(eps_t[:], EPS)
        bfc1_sb = const.tile([128, FCH], F32)
        bfc2_sb = const.tile([128, DC], F32)
        gb1_sb = const.tile([128, 2 * DC], F32)
        gb2_sb = const.tile([128, 2 * DC], F32)
        nc.sync.dma_start(out=bfc1_sb[:], in_=bfc1C[:])
        nc.sync.dma_start(out=bfc2_sb[:], in_=bfc2C[:])
        if not sc["gb1_fast"]:
            nc.sync.dma_start(out=gb1_sb[:], in_=gb1C[:])
        if not sc["gb2_fast"]:
            nc.sync.dma_start(out=gb2_sb[:], in_=gb2C[:])

        def bcast(dst_ap, src_ap, nfree):
            """partition-broadcast a [1, nfree] SBUF row via DRAM roundtrip"""
            scr = dscr.tile([nfree], F16, name="bscr")
            nc.sync.dma_start(out=scr[:], in_=src_ap)
            nc.sync.dma_start(
                out=dst_ap,
                in_=scr[:].unsqueeze(0).to_broadcast([dst_ap.shape[0], nfree]))

        # ---------------- tiles + input DMA ------------------------------
        xTb_sb = p_xTb.tile([128, DC, N], F32)
        ctx_sb = p_ctx.tile([128, DC, N], F8)
        wproj_sb = p_ctx.tile([128, DC, D], F8)
        q_sb = p_attn.tile([128, DC, N], F16)
        k2_sb = p_attn.tile([128, 2 * DC, N], F16)
        # per-mt row padded 780 -> 784 bytes: DoubleRow ldweights requires
        # the outer stationary stride to be 16-byte aligned
        VW = HEADS * (HD + 1) + 4
        v_sb = p_attn.tile([128, MT, VW], F8)

        def vv(mt_sl):
            return v_sb[:, mt_sl, 0:HEADS * (HD + 1)].rearrange(
                "p m (h e) -> p m h e", e=HD + 1)

        x8_sb = p_qkv_in.tile([128, DC, N], F8)
        wqkv_sb = p_qkv_in.tile([128, DC, 3 * D], F8)
        craw_sb = p_craw.tile([HD + 1, 4, 512], F32)
        w1_sb = p_w1.tile([128, DC, HID], F8 if fc1_fp8 else F16)

        # x8/wqkv-k chunk DMAs interleaved so the first k matmul can start
        # after ~0.5MB instead of the full prefetch
        for i in range(3):
            rs = slice(256 * i, 256 * i + 256)
            nc.sync.dma_start(
                out=x8_sb[:, 2 * i:2 * i + 2, :],
                in_=xT8[rs, :].rearrange("(c p) n -> p c n", p=128))
            nc.sync.dma_start(
                out=wqkv_sb[:, 2 * i:2 * i + 2, D:2 * D],
                in_=wqkvT8[rs, D:2 * D].rearrange("(c p) n -> p c n", p=128))
        nc.sync.dma_start(
            out=wqkv_sb[:, :, 0:D],
            in_=wqkvT8[:, 0:D].rearrange("(c p) n -> p c n", p=128))
        nc.sync.dma_start(
            out=wqkv_sb[:, :, 2 * D:3 * D],
            in_=wqkvT8[:, 2 * D:3 * D].rearrange("(c p) n -> p c n", p=128))
        nc.sync.dma_start(out=wproj_sb[:],
                          in_=wprojT8[:, :].rearrange("(c p) n -> p c n", p=128))
        nc.sync.dma_start(out=xTb_sb[:],
                          in_=xTb[:, :].rearrange("(c p) n -> p c n", p=128))
        nc.sync.dma_start(out=w1_sb[:],
                          in_=wfc1T[:, :].rearrange("(c p) n -> p c n", p=128))

        # zero halves for the head-pair packing of k; ones column for the
        # softmax denominators
        nc.vector.memset(k2_sb[64:128, 0:DC, :], 0.0)
        nc.vector.memset(k2_sb[0:64, DC:2 * DC, :], 0.0)
        nc.vector.memset(vv(slice(0, MT))[:, :, :, HD:HD + 1], 1.0)

        # ---------------- QKV projection machinery -----------------------
        ps_qk = tc.alloc_tile_pool(name="ps_qk", bufs=1, space="PSUM")
        ps_v = tc.alloc_tile_pool(name="ps_v", bufs=1, space="PSUM")

        # q/k PSUM evacuations run on the DVE (tensor_scalar applies the
        # fp8 descale) so the ACT engine is dedicated to exp.
        def qk_pieces_scaled(jt):
            """4 closures computing q/k block jt: 3 cp-pair DoubleRow
            matmul steps + the PSUM evacuation (DVE)."""
            st = {}

            def mm(cp):
                def go():
                    if "ps" not in st:
                        st["ps"] = ps_qk.tile([128, N], F32, tag="qk",
                                              name="psqk")
                    for nb in range(NB):
                        sl = slice(nb * 512, nb * 512 + 512)
                        nc.tensor.matmul(
                            st["ps"][:, sl],
                            wqkv_sb[:, cp:cp + 2, jt * 128:(jt + 1) * 128],
                            x8_sb[:, cp:cp + 2, sl],
                            start=(cp == 0), stop=(cp == DC - 2),
                            perf_mode=DR)
                return go

            def evac():
                ps = st["ps"]
                if jt < DC:
                    nc.vector.tensor_scalar_mul(q_sb[:, jt, :], in0=ps[:],
                                                scalar1=s_q)
                else:
                    j = jt - DC
                    nc.vector.tensor_scalar_mul(k2_sb[0:64, j, :],
                                                in0=ps[0:64, :], scalar1=s_k)
                    nc.vector.tensor_scalar_mul(k2_sb[64:128, DC + j, :],
                                                in0=ps[64:128, :],
                                                scalar1=s_k)
            return [mm(0), mm(2), mm(4), evac]

        # prelude: k/q for head pair 0, then v for all heads
        for piece in qk_pieces_scaled(DC) + qk_pieces_scaled(0):
            piece()
        for mt in range(MT):
            ps = ps_v.tile([128, D], F32, tag="v", name="psv")
            for cp in range(0, DC, 2):
                nc.tensor.matmul(ps[:, 0:512],
                                 x8_sb[:, cp:cp + 2, mt * 128:(mt + 1) * 128],
                                 wqkv_sb[:, cp:cp + 2, 2 * D:2 * D + 512],
                                 start=(cp == 0), stop=(cp == DC - 2),
                                 perf_mode=DR)
                nc.tensor.matmul(ps[:, 512:768],
                                 x8_sb[:, cp:cp + 2, mt * 128:(mt + 1) * 128],
                                 wqkv_sb[:, cp:cp + 2, 2 * D + 512:3 * D],
                                 start=(cp == 0), stop=(cp == DC - 2),
                                 perf_mode=DR)
            nc.vector.tensor_scalar_mul(
                vv(slice(mt, mt + 1))[:, 0, :, 0:HD],
                in0=ps[:].rearrange("p (h d) -> p h d", h=HEADS),
                scalar1=s_v)
        ps_v.release()

        # ---------------- attention (with QKV filler) --------------------
        ps_sc = tc.alloc_tile_pool(name="ps_sc", bufs=1, space="PSUM")
        ps_cp = tc.alloc_tile_pool(name="ps_cp", bufs=1, space="PSUM")

        for pr in range(HEADS // 2):
            filler = []
            if pr < HEADS // 2 - 1:
                filler = qk_pieces_scaled(DC + pr + 1) + \
                         qk_pieces_scaled(pr + 1)
            fi = iter(filler)
            ae = {h01: p_ae.tile([128, 2, N], F8, tag=f"ae{h01}", name="ae")
                  for h01 in range(2)}
            cps = {}
            for h01 in range(2):
                for nb in range(NB):
                    cps[(h01, nb)] = ps_cp.tile(
                        [HD + 1, 512], F32, tag=f"c{h01}{nb}", name="cps")
            for mt in range(MT):
                msl = slice(mt * 128, mt * 128 + 128)
                for h01 in range(2):
                    ps = ps_sc.tile([128, N], F32, tag="sc", name="pssc")
                    for nb in range(NB):
                        sl = slice(nb * 512, nb * 512 + 512)
                        nc.tensor.matmul(ps[:, sl],
                                         k2_sb[:, h01 * DC + pr, msl],
                                         q_sb[:, pr, sl],
                                         start=True, stop=True)
                    nc.scalar.activation(out=ae[h01][:, mt % 2, :],
                                         in_=ps[:], func=AF.Exp,
                                         bias=expb_t[:])
                    nxt = next(fi, None)
                    if nxt is not None:
                        nxt()
                if mt % 2 == 1:
                    for h01 in range(2):
                        for nb in range(NB):
                            sl = slice(nb * 512, nb * 512 + 512)
                            h = 2 * pr + h01
                            nc.tensor.matmul(
                                cps[(h01, nb)][:],
                                v_sb[:, mt - 1:mt + 1,
                                     h * (HD + 1):(h + 1) * (HD + 1)],
                                ae[h01][:, :, sl],
                                start=(mt == 1), stop=(mt == MT - 1),
                                perf_mode=DR)
            for piece in fi:
                piece()
            last = pr == HEADS // 2 - 1
            # stage context accumulators to SBUF so PSUM is free for the
            # next head pair; on the last pair read PSUM directly.
            if not last:
                for h01 in range(2):
                    for nb in range(NB):
                        nc.vector.tensor_copy(craw_sb[:, 2 * h01 + nb, :],
                                              cps[(h01, nb)][:])

            def crow(h01, nb, p0, p1):
                if last:
                    return cps[(h01, nb)][p0:p1, :]
                return craw_sb[p0:p1, 2 * h01 + nb, :]

            den4 = stats.tile([128, 512], F32, tag="den4", name="den4")
            rec4 = stats.tile([128, 512], F32, tag="rec4", name="rec4")
            rec4h = stats.tile([128, 512], F16, tag="rec4h", name="rec4h")
            if pr == 0:
                nc.vector.memset(den4[:], 1.0)  # benign filler rows
            for h01 in range(2):
                for nb in range(NB):
                    j = 32 * (2 * h01 + nb)  # DVE writes need 32-alignment
                    nc.vector.tensor_copy(den4[j:j + 1, :],
                                          crow(h01, nb, HD, HD + 1))
            nc.vector.reciprocal(rec4[:], den4[:])
            nc.vector.tensor_scalar_mul(rec4h[:], in0=rec4[:], scalar1=s_ctx)
            for h01 in range(2):
                half = h01 * 64
                for nb in range(NB):
                    sl = slice(nb * 512, nb * 512 + 512)
                    j = 32 * (2 * h01 + nb)
                    recb = bc.tile([64, 512], F16, tag="recb", name="recb")
                    bcast(recb[:], rec4h[j:j + 1, :], 512)
                    nc.vector.tensor_mul(ctx_sb[half:half + 64, pr, sl],
                                         crow(h01, nb, 0, HD), recb[:])
        ps_cp.release()
        ps_sc.release()
        ps_qk.release()
        p_ae.release()
        p_craw.release()
        p_qkv_in.release()
        p_attn.release()

        # ---------------- proj + residual, LN1 ---------------------------
        p_w2 = tc.alloc_tile_pool(name="p_w2", bufs=1)
        p_x116 = tc.alloc_tile_pool(name="p_x116", bufs=1)
        p_sq = tc.alloc_tile_pool(name="p_sq", bufs=2)
        p_r1 = tc.alloc_tile_pool(name="p_r1", bufs=1)
        ps_ln = tc.alloc_tile_pool(name="ps_ln", bufs=1, space="PSUM")
        ps_bc = tc.alloc_tile_pool(name="ps_bc", bufs=1, space="PSUM")
        ps_pj = tc.alloc_tile_pool(name="ps_pj", bufs=2, space="PSUM")
        w2_sb = p_w2.tile([128, FCH, D], F16)
        nc.sync.dma_start(out=w2_sb[:],
                          in_=wfc2T[:, :].rearrange("(c p) n -> p c n", p=128))
        r1_sb = p_r1.tile([128, DC, N], F16)
        x116_sb = p_x116.tile([128, DC, N], F16)
        x18_sb = (p_x116.tile([128, DC, N], F8, name="x18")
                  if fc1_fp8 else None)

        for et in range(DC):
            ps = ps_pj.tile([128, N], F32, tag="pj", name="pspj")
            for cp in range(0, DC, 2):
                for nb in range(NB):
                    sl = slice(nb * 512, nb * 512 + 512)
                    nc.tensor.matmul(ps[:, sl],
                                     wproj_sb[:, cp:cp + 2,
                                              et * 128:(et + 1) * 128],
                                     ctx_sb[:, cp:cp + 2, sl],
                                     start=(cp == 0), stop=(cp == DC - 2),
                                     perf_mode=DR)
            nc.vector.scalar_tensor_tensor(
                out=r1_sb[:, et, :], in0=ps[:], scalar=s_pj,
                in1=xTb_sb[:, et, :], op0=OP.mult, op1=OP.add)
        ps_pj.release()

        def layer_norm16(src_sb, out_sb, gb_fast, gb_sb, nb, out_sl=None,
                         out8_sb=None, out8_scale=1.0):
            """LN over features for token block nb; src fp16 [128, DC, N]."""
            sl = slice(nb * 512, nb * 512 + 512)
            osl = sl if out_sl is None else out_sl
            s1 = ps_ln.tile([1, 512], F32, tag="s1", name="s1")
            s2 = ps_ln.tile([1, 512], F32, tag="s2", name="s2")
            for c in range(DC):
                nc.tensor.matmul(s1[:], ones16[:], src_sb[:, c, sl],
                                 start=(c == 0), stop=(c == DC - 1))
            for c in range(DC):
                sq = p_sq.tile([128, 512], F16, tag="sq", name="sq")
                nc.vector.tensor_mul(sq[:], src_sb[:, c, sl], src_sb[:, c, sl])
                nc.tensor.matmul(s2[:], ones16[:], sq[:],
                                 start=(c == 0), stop=(c == DC - 1))
            t0 = stats.tile([1, 512], F32, tag="t0", name="t0")
            m2 = stats.tile([1, 512], F32, tag="m2", name="m2")
            var = stats.tile([1, 512], F32, tag="var", name="var")
            lnv = stats.tile([1, 512], F32, tag="lnv", name="lnv")
            a16 = stats.tile([1, 512], F16, tag="a16", name="a16")
            b16 = stats.tile([1, 512], F16, tag="b16", name="b16")
            nc.vector.tensor_scalar_mul(t0[:], in0=s1[:], scalar1=1.0 / D)
            nc.vector.tensor_mul(m2[:], t0[:], t0[:])
            nc.vector.scalar_tensor_tensor(out=var[:], in0=s2[:],
                                           scalar=1.0 / D, in1=m2[:],
                                           op0=OP.mult, op1=OP.subtract)
            # 1/sqrt(var+eps) = exp(-0.5*ln(var+eps)): Ln and Exp share one
            # ACT table, so this costs no table reload next to the softmax
            nc.scalar.activation(out=lnv[:], in_=var[:], func=AF.Ln,
                                 bias=eps_t[:])
            nc.scalar.activation(out=a16[:], in_=lnv[:], func=AF.Exp,
                                 scale=-0.5)
            nc.vector.scalar_tensor_tensor(out=b16[:], in0=a16[:],
                                           scalar=-1.0, in1=t0[:],
                                           op0=OP.mult, op1=OP.mult)
            A_ps = ps_bc.tile([128, 512], F32, tag="A", name="Aps")
            B_ps = ps_bc.tile([128, 512], F32, tag="B", name="Bps")
            nc.tensor.matmul(A_ps[:], ones_row16[:], a16[:],
                             start=True, stop=True)
            nc.tensor.matmul(B_ps[:], ones_row16[:], b16[:],
                             start=True, stop=True)
            A = bc.tile([128, 512], F16, tag="A", name="A")
            B = bc.tile([128, 512], F16, tag="B", name="B")
            nc.scalar.activation(out=A[:], in_=A_ps[:], func=AF.Copy)
            nc.scalar.activation(out=B[:], in_=B_ps[:], func=AF.Copy)
            for c in range(DC):
                u = p_sq.tile([128, 512], F16, tag="u", name="u")
                nc.vector.tensor_mul(u[:], src_sb[:, c, sl], A[:])
                if gb_fast:
                    nc.vector.tensor_add(out_sb[:, c, osl], u[:], B[:])
                else:
                    w = p_sq.tile([128, 512], F16, tag="w", name="w")
                    nc.vector.tensor_add(w[:], u[:], B[:])
                    nc.vector.tensor_scalar(
                        out=out_sb[:, c, osl], in0=w[:],
                        scalar1=gb_sb[:, c:c + 1],
                        scalar2=gb_sb[:, DC + c:DC + c + 1],
                        op0=OP.mult, op1=OP.add)
                if out8_sb is not None:
                    nc.vector.tensor_scalar_mul(out8_sb[:, c, sl],
                                                in0=out_sb[:, c, osl],
                                                scalar1=out8_scale)

        x18_scale = 2.0 ** sc["kx1"] if fc1_fp8 else 1.0
        layer_norm16(r1_sb, x116_sb, sc["gb1_fast"], gb1_sb, 0,
                     out8_sb=x18_sb, out8_scale=x18_scale)
        layer_norm16(r1_sb, x116_sb, sc["gb1_fast"], gb1_sb, 1,
                     out8_sb=x18_sb, out8_scale=x18_scale)
        p_ctx.release()
        p_xTb.release()
        p_r1.release()

        # ---------------- MLP (+ residual), LN2, output ------------------
        p_h = tc.alloc_tile_pool(name="p_h", bufs=1)
        p_y2 = tc.alloc_tile_pool(name="p_y2", bufs=1)
        p_x2 = tc.alloc_tile_pool(name="p_x2", bufs=1)
        h_sb = p_h.tile([128, FCH, N], F16)
        y2_sb = p_y2.tile([128, DC, N], F16)
        x2_sb = p_x2.tile([128, DC, 512], F32)
        ps_f1 = tc.alloc_tile_pool(name="ps_f1", bufs=2, space="PSUM")
        ps_f2 = tc.alloc_tile_pool(name="ps_f2", bufs=2, space="PSUM")

        def fc1(nb):
            sl = slice(nb * 512, nb * 512 + 512)
            for f in range(FCH):
                ps = ps_f1.tile([128, 512], F32, tag="f1", name="psf1")
                if fc1_fp8:
                    for cp in range(0, DC, 2):
                        nc.tensor.matmul(ps[:],
                                         w1_sb[:, cp:cp + 2,
                                               f * 128:(f + 1) * 128],
                                         x18_sb[:, cp:cp + 2, sl],
                                         start=(cp == 0),
                                         stop=(cp == DC - 2), perf_mode=DR)
                else:
                    for c in range(DC):
                        nc.tensor.matmul(ps[:],
                                         w1_sb[:, c, f * 128:(f + 1) * 128],
                                         x116_sb[:, c, sl],
                                         start=(c == 0), stop=(c == DC - 1))
                nc.scalar.activation(out=h_sb[:, f, sl], in_=ps[:],
                                     func=AF.Gelu, scale=s_f1,
                                     bias=bfc1_sb[:, f:f + 1])

        def fc2(nb):
            sl = slice(nb * 512, nb * 512 + 512)
            for et in range(DC):
                ps = ps_f2.tile([128, 512], F32, tag="f2", name="psf2")
                for f in range(FCH):
                    nc.tensor.matmul(ps[:],
                                     w2_sb[:, f, et * 128:(et + 1) * 128],
                                     h_sb[:, f, sl],
                                     start=(f == 0), stop=(f == FCH - 1))
                nc.vector.scalar_tensor_tensor(
                    out=y2_sb[:, et, sl], in0=ps[:],
                    scalar=bfc2_sb[:, et:et + 1], in1=x116_sb[:, et, sl],
                    op0=OP.add, op1=OP.add)

        fc1(0)
        fc1(1)
        fc2(0)
        layer_norm16(y2_sb, x2_sb, sc["gb2_fast"], gb2_sb, 0,
                     out_sl=slice(0, 512))
        for c in range(DC):
            nc.sync.dma_start(out=yT[c * 128:(c + 1) * 128, 0:512],
                              in_=x2_sb[:, c, :])
        fc2(1)
        layer_norm16(y2_sb, x2_sb, sc["gb2_fast"], gb2_sb, 1,
                     out_sl=slice(0, 512))
        for c in range(DC):
            nc.sync.dma_start(out=yT[c * 128:(c + 1) * 128, 512:1024],
                              in_=x2_sb[:, c, :])

        ps_f2.release()
        ps_f1.release()
        ps_pj_ = None
        ps_bc.release()
        ps_ln.release()
        p_x2.release()
        p_y2.release()
        p_h.release()
        p_r1_ = None  # r1 released after LN1
        p_sq.release()
        p_x116.release()
        p_w2.release()
        dscr.release()
        bc.release()
        stats.release()
        p_w1.release()
        const.release()
    return nc


_NC_CACHE = {}


def _get_nc(sc):
    key = tuple(sorted(sc.items()))
    if key not in _NC_CACHE:
        nc = _build(sc)
        _split_excess_waits(nc)
        _NC_CACHE.clear()
        _NC_CACHE[key] = nc
    return _NC_CACHE[key]


def _kexp(amax, target=120.0):
    """power-of-2 scale exponent: amax * 2^k ~= target (<= 240)"""
    return int(np.floor(np.log2(target / max(amax, 1e-30))))


def _q8(a, k):
    return np.clip(a * (2.0 ** k), -240.0, 240.0).astype(NP_F8)


def kernel(x, w_qkv, w_proj, b_proj, w_fc1, b_fc1, w_fc2, b_fc2,
           gamma1, beta1, gamma2, beta2):
    global LAST_RESULT
    x = np.asarray(x, dtype=np.float32)
    w_qkv = np.asarray(w_qkv, dtype=np.float32)
    w_proj = np.asarray(w_proj, dtype=np.float32)
    b_proj = np.asarray(b_proj, dtype=np.float32)
    w_fc1 = np.asarray(w_fc1, dtype=np.float32)
    b_fc1 = np.asarray(b_fc1, dtype=np.float32)
    w_fc2 = np.asarray(w_fc2, dtype=np.float32)
    b_fc2 = np.asarray(b_fc2, dtype=np.float32)
    gamma1 = np.asarray(gamma1, dtype=np.float32)
    beta1 = np.asarray(beta1, dtype=np.float32)
    gamma2 = np.asarray(gamma2, dtype=np.float32)
    beta2 = np.asarray(beta2, dtype=np.float32)

    wq = w_qkv.copy()
    wq[:D] *= HD ** -0.5                     # fold attention scale into Q

    # host-side range probe (fp32, BLAS) to pick exact power-of-2 fp8 scales
    xf = x.reshape(-1, D)
    qh = (xf @ wq[:D].T).reshape(NCORES, N, HEADS, HD)
    kh = (xf @ wq[D:2 * D].T).reshape(NCORES, N, HEADS, HD)
    vh = (xf @ wq[2 * D:].T).reshape(NCORES, N, HEADS, HD)
    smax = 0.0
    cmax = 0.0
    ctxs = np.empty((NCORES, N, HEADS, HD), np.float32)
    for b in range(NCORES):
        for h in range(HEADS):
            s = qh[b, :, h] @ kh[b, :, h].T
            smax = max(smax, float(np.abs(s).max()))
            p = np.exp(s - s.max(axis=-1, keepdims=True))
            cn = (p @ vh[b, :, h]) / p.sum(axis=-1, keepdims=True)
            ctxs[b, :, h] = cn
            cmax = max(cmax, float(np.abs(cn).max()))
    # x1 absmax for the fc1 fp8 input scale
    attn_out = ctxs.reshape(-1, DIM_ := D) @ w_proj.T + b_proj
    r1 = xf + attn_out
    mu = r1.mean(-1, keepdims=True)
    vr = ((r1 - mu) ** 2).mean(-1, keepdims=True)
    x1 = (r1 - mu) / np.sqrt(vr + EPS) * gamma1 + beta1
    x1max = float(np.abs(x1).max())

    sc = {
        "kx": _kexp(np.abs(x).max()),
        "kq": _kexp(np.abs(wq[:D]).max()),
        "kk": _kexp(np.abs(wq[D:2 * D]).max()),
        "kvw": _kexp(np.abs(wq[2 * D:]).max()),
        "kv": _kexp(np.abs(vh).max()),
        "kp": int(np.floor(np.log2(120.0 / np.exp(smax)))),
        "kc": _kexp(cmax),
        "kpr": _kexp(np.abs(w_proj).max()),
        "kx1": _kexp(x1max),
        "kw1": _kexp(np.abs(w_fc1).max()),
        "fc1_fp8": FC1_FP8,
        "gb1_fast": bool(np.all(gamma1 == 1.0) and np.all(beta1 == 0.0)),
        "gb2_fast": bool(np.all(gamma2 == 1.0) and np.all(beta2 == 0.0)),
    }

    wqkv8 = np.concatenate([
        _q8(wq[:D], sc["kq"]), _q8(wq[D:2 * D], sc["kk"]),
        _q8(wq[2 * D:], sc["kvw"])], axis=0)
    wqkvT8 = np.ascontiguousarray(wqkv8.T)
    wprojT8 = np.ascontiguousarray(_q8(w_proj, sc["kpr"]).T)
    if FC1_FP8:
        wfc1T = np.ascontiguousarray(_q8(w_fc1, sc["kw1"]).T)
    else:
        wfc1T = np.ascontiguousarray(w_fc1.T.astype(np.float16))
    wfc2T = np.ascontiguousarray(w_fc2.T.astype(np.float16))

    def cols(v, nchunks):
        return np.ascontiguousarray(v.reshape(nchunks, 128).T)

    shared = {
        "wqkvT8": wqkvT8, "wprojT8": wprojT8,
        "wfc1T": wfc1T, "wfc2T": wfc2T,
        "bfc1C": cols(b_fc1, FCH), "bfc2C": cols(b_fc2, DC),
        "gb1C": np.concatenate([cols(gamma1, DC), cols(beta1, DC)], 1),
        "gb2C": np.concatenate([cols(gamma2, DC), cols(beta2, DC)], 1),
    }
    in_maps = []
    for b in range(NCORES):
        m = dict(shared)
        xt = np.ascontiguousarray(x[b].T)
        m["xTb"] = xt + b_proj[:, None]
        m["xT8"] = _q8(xt, sc["kx"])
        in_maps.append(m)

    nc = _get_nc(sc)
    LAST_RESULT = run_bass_kernel_spmd(nc, in_maps, list(range(NCORES)))
    out = np.stack([np.ascontiguousarray(LAST_RESULT.results[b]["yT"].T)
                    for b in range(NCORES)])
    return out.astype(np.float32)


# revision 72
# speedup vs baseline: 1.0959x; 1.0959x over previous
"""Trainium2 Bass kernel for a prenorm transformer Block (B=8, N=1024, D=768,
12 heads, MLP hidden 3072), data-parallel over batch across 8 NeuronCores.

Layout: activations transposed on-device (features on partitions, tokens on
the free dim) so the whole chain runs without on-device transposes.

Design highlights (v3):
  - QKV / attention-context / proj / fc1 matmuls run in fp8e4m3 DoubleRow
    (two 128-row contraction chunks per instruction, 2x PE throughput).
    All fp8 scales are exact powers of two picked on the host from the
    actual inputs; descales fold into existing PSUM-evacuation ops.
    fc2 stays fp16 (fp8 there costs ~1.3e-2 rel err on its own).
  - The attention window is Activation-engine bound (exp of all 12.6M
    scores).  QKV projection work for head-pair p+1 is software-pipelined
    INTO head-pair p's score/exp loop as PE filler, which both hides the
    QKV phase entirely and keeps the PE streaming (avoiding its
    half-clock pstate after stalls).  All QKV PSUM evacuations run on the
    DVE so the ACT engine does nothing but exp.
  - Softmax: scoresT layout, exp with a +kp*ln2 bias (fp8 output), ones
    column on V for denominators, context accumulators staged to SBUF
    immediately (frees PSUM for the next head pair), denominator rows
    batch-inverted with one DVE reciprocal per head pair, partition
    broadcast via DRAM roundtrip off the critical path.
  - fp16 residual stream; LayerNorm stats via PE ones-matmuls; the
    1/sqrt(var+eps) uses exp(-0.5*ln(var+eps)) on ACT (Ln and Exp share
    one table with the softmax exp, so no table reloads); per-token
    scale/shift rows broadcast on the PE into PSUM, affine is two
    2x-mode fp16 DVE passes.
  - fc2 accumulates over all 24 hidden chunks in PSUM; residual + bias
    via one fused scalar_tensor_tensor.
  - gamma/beta fold away when they are ones/zeros (checked on host at
    build time; a generic tensor_scalar pass is emitted otherwise).
"""
import sys
import types

sys.path.insert(0, "/opt/trn_rl_repo")

# concourse.bass_utils imports antenv.axon_hooks when tracing is requested;
# provide a no-op registry if the container image lacks that module so a
# BASS_TRACE=1 environment degrades to "no trace" instead of crashing.
try:
    import antenv.axon_hooks  # noqa: F401
except Exception:
    try:
        import antenv

        _hooks = types.ModuleType("antenv.axon_hooks")
        _hooks._hook = None

        def _set_hook(h):
            _hooks._hook = h

        def _get_hook():
            return _hooks._hook

        _hooks.set_axon_ntff_profile_hook = _set_hook
        _hooks.get_axon_ntff_profile_hook = _get_hook
        sys.modules["antenv.axon_hooks"] = _hooks
        antenv.axon_hooks = _hooks
    except Exception:
        pass

# boot() registers the NTFF profile hook only if antenv.axon_hooks exists at
# interpreter start; on this image it doesn't, so register it here through the
# shim so BASS_TRACE=1 yields exec times + perfetto traces.
try:
    import antenv.axon_hooks as _ah

    if _ah.get_axon_ntff_profile_hook() is None:
        from trn_agent_boot.trn_boot import _ntff_profile_via_ctypes

        _hk = _ntff_profile_via_ctypes("/opt/axon/libaxon_pjrt.so")
        if _hk is not None:
            _ah.set_axon_ntff_profile_hook(_hk)
except Exception:
    pass

import math

import ml_dtypes
import numpy as np

import concourse.bass as bass
import concourse.tile as tile
from concourse import mybir
from concourse.bass_utils import run_bass_kernel_spmd

F32 = mybir.dt.float32
F16 = mybir.dt.float16
F8 = mybir.dt.float8e4
AF = mybir.ActivationFunctionType
OP = mybir.AluOpType
DR = mybir.MatmulPerfMode.DoubleRow
NP_F8 = ml_dtypes.float8_e4m3  # TRN FP8_EXP4: max +-240

NCORES = 8
D, HEADS, HID, N = 768, 12, 3072, 1024
HD = D // HEADS                  # 64 head dim
DC = D // 128                    # 6 feature chunks
NB = N // 512                    # 2 moving-dim blocks
MT = N // 128                    # 8 key tiles
FCH = HID // 128                 # 24 hidden chunks
EPS = 1e-6
FC1_FP8 = True                   # fc1 in fp8 DoubleRow (fc2 stays fp16)

LAST_RESULT = None               # BassKernelResults of the most recent run


# The walrus build in this container rejects instructions carrying more than
# a couple of sync waits ("Too many sync wait commands"); fp8/fp16 matmuls
# reject more than one. Excess waits are hoisted onto standalone
# EventSemaphore carriers placed right before the instruction on the same
# engine, which is semantically identical (waits gate the engine stream).
_MM_OPS = ("Matmult", "Ldweights")


def _split_excess_waits(nc, default_limit=1, matmul_limit=0):
    counter = 0
    for f in nc.m.functions:
        for bb in f.blocks:
            new_insts = []
            for inst in bb.instructions:
                si = inst.sync_info
                waits = list(si.on_wait) if si and si.on_wait else []
                limit = matmul_limit if inst.opcode in _MM_OPS else default_limit
                if len(waits) > limit:
                    keep, move = waits[:limit], waits[limit:]
                    for w in move:
                        counter += 1
                        ev = mybir.InstEventSemaphore(
                            name=f"I-waitsplit-{counter}",
                            engine=inst.engine,
                            sync_info=mybir.SyncInfo(on_wait=[w], on_update=[]),
                        )
                        nc.register_instruction(ev, overwrite=True)
                        new_insts.append(ev)
                    inst.sync_info = mybir.SyncInfo(
                        on_wait=keep, on_update=list(si.on_update) if si else []
                    )
                new_insts.append(inst)
            bb.instructions = new_insts
    return counter


def _build(sc):
    """sc: dict of integer scale exponents + gamma/beta fast-path flags."""
    nc = bass.Bass()

    fc1_fp8 = sc["fc1_fp8"]
    xTb = nc.dram_tensor("xTb", [D, N], F32, kind="ExternalInput")
    xT8 = nc.dram_tensor("xT8", [D, N], F8, kind="ExternalInput")
    wqkvT8 = nc.dram_tensor("wqkvT8", [D, 3 * D], F8, kind="ExternalInput")
    wprojT8 = nc.dram_tensor("wprojT8", [D, D], F8, kind="ExternalInput")
    wfc1T = nc.dram_tensor("wfc1T", [D, HID], F8 if fc1_fp8 else F16,
                           kind="ExternalInput")
    wfc2T = nc.dram_tensor("wfc2T", [HID, D], F16, kind="ExternalInput")
    bfc1C = nc.dram_tensor("bfc1C", [128, FCH], F32, kind="ExternalInput")
    bfc2C = nc.dram_tensor("bfc2C", [128, DC], F32, kind="ExternalInput")
    gb1C = nc.dram_tensor("gb1C", [128, 2 * DC], F32, kind="ExternalInput")
    gb2C = nc.dram_tensor("gb2C", [128, 2 * DC], F32, kind="ExternalInput")
    yT = nc.dram_tensor("yT", [D, N], F32, kind="ExternalOutput")

    s_q = 2.0 ** (-sc["kx"] - sc["kq"])          # psum -> true q
    s_k = 2.0 ** (-sc["kx"] - sc["kk"])
    s_v = 2.0 ** (sc["kv"] - sc["kx"] - sc["kvw"])   # psum -> 2^kv * v
    s_ctx = 2.0 ** (sc["kc"] - sc["kv"])             # craw -> 2^kc * ctx
    s_pj = 2.0 ** (-sc["kc"] - sc["kpr"])            # psum -> true attn_out
    s_f1 = 2.0 ** (-sc["kx1"] - sc["kw1"]) if fc1_fp8 else 1.0
    exp_bias = float(sc["kp"] * math.log(2.0))       # exp(s + kp ln2)

    with tile.TileContext(nc) as tc:
        # ---------------- pools ----------------
        const = tc.alloc_tile_pool(name="const", bufs=1)
        p_w1 = tc.alloc_tile_pool(name="p_w1", bufs=1)
        stats = tc.alloc_tile_pool(name="stats", bufs=1)
        bc = tc.alloc_tile_pool(name="bc", bufs=2)
        p_xTb = tc.alloc_tile_pool(name="p_xTb", bufs=1, side="right")
        p_ctx = tc.alloc_tile_pool(name="p_ctx", bufs=1, side="right")
        p_attn = tc.alloc_tile_pool(name="p_attn", bufs=1, side="right")
        p_qkv_in = tc.alloc_tile_pool(name="p_qkv_in", bufs=1, side="right")
        p_craw = tc.alloc_tile_pool(name="p_craw", bufs=1, side="right")
        p_ae = tc.alloc_tile_pool(name="p_ae", bufs=2, side="right")
        dscr = tc.alloc_tile_pool(name="dscr", bufs=4, space="DRAM")

        ones_row16 = const.tile([1, 128], F16)
        nc.vector.tensor_copy(ones_row16[:], nc.const_aps.tensor(1.0, (1, 128)))
        ones16 = const.tile([128, 1], F16)
        nc.vector.tensor_copy(ones16[:], nc.const_aps.tensor(1.0, (128, 1)))
        expb_t = const.tile([128, 1], F32)
        nc.vector.memset(expb_t[:], exp_bias)
        eps_t = const.tile([1, 1], F32)
        nc.vector.memset(eps_t[:], EPS)
        bfc1_sb = const.tile([128, FCH], F32)
        bfc2_sb = const.tile([128, DC], F32)
        gb1_sb = const.tile([128, 2 * DC], F32)
        gb2_sb = const.tile([128, 2 * DC], F32)
        nc.sync.dma_start(out=bfc1_sb[:], in_=bfc1C[:])
        nc.sync.dma_start(out=bfc2_sb[:], in_=bfc2C[:])
        if not sc["gb1_fast"]:
            nc.sync.dma_start(out=gb1_sb[:], in_=gb1C[:])
        if not sc["gb2_fast"]:
            nc.sync.dma_start(out=gb2_sb[:], in_=gb2C[:])

        def bcast(dst_ap, src_ap, nfree):
            """partition-broadcast a [1, nfree] SBUF row via DRAM roundtrip"""
            scr = dscr.tile([nfree], F16, name="bscr")
            nc.sync.dma_start(out=scr[:], in_=src_ap)
            nc.sync.dma_start(
                out=dst_ap,
                in_=scr[:].unsqueeze(0).to_broadcast([dst_ap.shape[0], nfree]))

        # ---------------- tiles + input DMA ------------------------------
        xTb_sb = p_xTb.tile([128, DC, N], F32)
        ctx_sb = p_ctx.tile([128, DC, N], F8)
        wproj_sb = p_ctx.tile([128, DC, D], F8)
        q_sb = p_attn.tile([128, DC, N], F16)
        k2_sb = p_attn.tile([128, 2 * DC, N], F16)
        # per-mt row padded 780 -> 784 bytes: DoubleRow ldweights requires
        # the outer stationary stride to be 16-byte aligned
        VW = HEADS * (HD + 1) + 4
        v_sb = p_attn.tile([128, MT, VW], F8)

        def vv(mt_sl):
            return v_sb[:, mt_sl, 0:HEADS * (HD + 1)].rearrange(
                "p m (h e) -> p m h e", e=HD + 1)

        x8_sb = p_qkv_in.tile([128, DC, N], F8)
        wqkv_sb = p_qkv_in.tile([128, DC, 3 * D], F8)
        craw_sb = p_craw.tile([HD + 1, 4, 512], F32)
        w1_sb = p_w1.tile([128, DC, HID], F8 if fc1_fp8 else F16)

        # x8/wqkv-k chunk DMAs interleaved so the first k matmul can start
        # after ~0.5MB instead of the full prefetch
        for i in range(3):
            rs = slice(256 * i, 256 * i + 256)
            nc.sync.dma_start(
                out=x8_sb[:, 2 * i:2 * i + 2, :],
                in_=xT8[rs, :].rearrange("(c p) n -> p c n", p=128))
            nc.sync.dma_start(
                out=wqkv_sb[:, 2 * i:2 * i + 2, D:2 * D],
                in_=wqkvT8[rs, D:2 * D].rearrange("(c p) n -> p c n", p=128))
        nc.sync.dma_start(
            out=wqkv_sb[:, :, 0:D],
            in_=wqkvT8[:, 0:D].rearrange("(c p) n -> p c n", p=128))
        nc.sync.dma_start(
            out=wqkv_sb[:, :, 2 * D:3 * D],
            in_=wqkvT8[:, 2 * D:3 * D].rearrange("(c p) n -> p c n", p=128))
        nc.sync.dma_start(out=wproj_sb[:],
                          in_=wprojT8[:, :].rearrange("(c p) n -> p c n", p=128))
        nc.sync.dma_start(out=xTb_sb[:],
                          in_=xTb[:, :].rearrange("(c p) n -> p c n", p=128))
        nc.sync.dma_start(out=w1_sb[:],
                          in_=wfc1T[:, :].rearrange("(c p) n -> p c n", p=128))

        # zero halves for the head-pair packing of k; ones column for the
        # softmax denominators
        nc.vector.memset(k2_sb[64:128, 0:DC, :], 0.0)
        nc.vector.memset(k2_sb[0:64, DC:2 * DC, :], 0.0)
        nc.vector.memset(vv(slice(0, MT))[:, :, :, HD:HD + 1], 1.0)

        # ---------------- QKV projection machinery -----------------------
        ps_qk = tc.alloc_tile_pool(name="ps_qk", bufs=1, space="PSUM")
        ps_v = tc.alloc_tile_pool(name="ps_v", bufs=1, space="PSUM")

        # q/k PSUM evacuations run on the DVE (tensor_scalar applies the
        # fp8 descale) so the ACT engine is dedicated to exp.
        def qk_pieces_scaled(jt):
            """4 closures computing q/k block jt: 3 cp-pair DoubleRow
            matmul steps + the PSUM evacuation (DVE)."""
            st = {}

            def mm(cp):
                def go():
                    if "ps" not in st:
                        st["ps"] = ps_qk.tile([128, N], F32, tag="qk",
                                              name="psqk")
                    for nb in range(NB):
                        sl = slice(nb * 512, nb * 512 + 512)
                        nc.tensor.matmul(
                            st["ps"][:, sl],
                            wqkv_sb[:, cp:cp + 2, jt * 128:(jt + 1) * 128],
                            x8_sb[:, cp:cp + 2, sl],
                            start=(cp == 0), stop=(cp == DC - 2),
                            perf_mode=DR)
                return go

            def evac():
                ps = st["ps"]
                if jt < DC:
                    nc.vector.tensor_scalar_mul(q_sb[:, jt, :], in0=ps[:],
                                                scalar1=s_q)
                else:
                    j = jt - DC
                    nc.vector.tensor_scalar_mul(k2_sb[0:64, j, :],
                                                in0=ps[0:64, :], scalar1=s_k)
                    nc.vector.tensor_scalar_mul(k2_sb[64:128, DC + j, :],
                                                in0=ps[64:128, :],
                                                scalar1=s_k)
            return [mm(0), mm(2), mm(4), evac]

        # prelude: k/q for head pair 0, then v for all heads
        for piece in qk_pieces_scaled(DC) + qk_pieces_scaled(0):
            piece()
        for mt in range(MT):
            ps = ps_v.tile([128, D], F32, tag="v", name="psv")
            for cp in range(0, DC, 2):
                nc.tensor.matmul(ps[:, 0:512],
                                 x8_sb[:, cp:cp + 2, mt * 128:(mt + 1) * 128],
                                 wqkv_sb[:, cp:cp + 2, 2 * D:2 * D + 512],
                                 start=(cp == 0), stop=(cp == DC - 2),
                                 perf_mode=DR)
                nc.tensor.matmul(ps[:, 512:768],
                                 x8_sb[:, cp:cp + 2, mt * 128:(mt + 1) * 128],
                                 wqkv_sb[:, cp:cp + 2, 2 * D + 512:3 * D],
                                 start=(cp == 0), stop=(cp == DC - 2),
                                 perf_mode=DR)
            nc.vector.tensor_scalar_mul(
                vv(slice(mt, mt + 1))[:, 0, :, 0:HD],
                in0=ps[:].rearrange("p (h d) -> p h d", h=HEADS),
                scalar1=s_v)
        ps_v.release()

        # ---------------- attention (with QKV filler) --------------------
        # Heads are processed INDIVIDUALLY (not in pairs) so the context
        # accumulator needs only 2 PSUM banks, which leaves room for
        # double-buffered [128, N] score tiles AND the interleaved QKV
        # projection pool: sc 4 + cps 2 + qk 2 = 8 banks.
        ps_sc = tc.alloc_tile_pool(name="ps_sc", bufs=2, space="PSUM")
        ps_cp = tc.alloc_tile_pool(name="ps_cp", bufs=1, space="PSUM")

        filler_blocks = [qk_pieces_scaled(DC + j) + qk_pieces_scaled(j)
                         for j in range(1, DC)]

        def crow(cps, h01, nb, p0, p1, direct):
            if direct:
                return cps[nb][p0:p1, :]
            return craw_sb[p0:p1, 2 * h01 + nb, :]

        for h in range(HEADS):
            pr, h01 = h // 2, h % 2
            fi = iter(filler_blocks[pr] if (h01 == 0 and pr + 1 < DC)
                      else ())
            ae = p_ae.tile([128, 2, N], F8, tag="ae", name="ae")
            cps = {nb: ps_cp.tile([HD + 1, 512], F32, tag=f"c{nb}",
                                  name="cps") for nb in range(NB)}
            for mt in range(MT):
                msl = slice(mt * 128, mt * 128 + 128)
                ps = ps_sc.tile([128, N], F32, tag="sc", name="pssc")
                for nb in range(NB):
                    sl = slice(nb * 512, nb * 512 + 512)
                    nc.tensor.matmul(ps[:, sl],
                                     k2_sb[:, h01 * DC + pr, msl],
                                     q_sb[:, pr, sl],
                                     start=True, stop=True)
                nc.scalar.activation(out=ae[:, mt % 2, :],
                                     in_=ps[:], func=AF.Exp,
                                     bias=expb_t[:])
                nxt = next(fi, None)
                if nxt is not None:
                    nxt()
                if mt % 2 == 1:
                    for nb in range(NB):
                        sl = slice(nb * 512, nb * 512 + 512)
                        nc.tensor.matmul(
                            cps[nb][:],
                            v_sb[:, mt - 1:mt + 1,
                                 h * (HD + 1):(h + 1) * (HD + 1)],
                            ae[:, :, sl],
                            start=(mt == 1), stop=(mt == MT - 1),
                            perf_mode=DR)
            for piece in fi:
                piece()
            last = h == HEADS - 1
            # stage the context accumulator to SBUF so its PSUM banks are
            # free for the next head; the last head reads PSUM directly.
            if not last:
                for nb in range(NB):
                    nc.vector.tensor_copy(craw_sb[:, 2 * h01 + nb, :],
                                          cps[nb][:])
            if h01 == 0:
                continue
            # softmax normalize for the completed head pair: batch the 4
            # denominator rows into one DVE reciprocal (its cost is per
            # free element), broadcast via DRAM roundtrip off the
            # critical path.
            den4 = stats.tile([128, 512], F32, tag="den4", name="den4")
            rec4 = stats.tile([128, 512], F32, tag="rec4", name="rec4")
            rec4h = stats.tile([128, 512], F16, tag="rec4h", name="rec4h")
            if pr == 0:
                nc.vector.memset(den4[:], 1.0)  # benign filler rows
            for hh in range(2):
                for nb in range(NB):
                    j = 32 * (2 * hh + nb)  # DVE writes need 32-alignment
                    nc.vector.tensor_copy(
                        den4[j:j + 1, :],
                        crow(cps, hh, nb, HD, HD + 1, last and hh == 1))
            nc.vector.reciprocal(rec4[:], den4[:])
            nc.vector.tensor_scalar_mul(rec4h[:], in0=rec4[:], scalar1=s_ctx)
            for hh in range(2):
                half = hh * 64
                for nb in range(NB):
                    sl = slice(nb * 512, nb * 512 + 512)
                    j = 32 * (2 * hh + nb)
                    recb = bc.tile([64, 512], F16, tag="recb", name="recb")
                    bcast(recb[:], rec4h[j:j + 1, :], 512)
                    nc.vector.tensor_mul(
                        ctx_sb[half:half + 64, pr, sl],
                        crow(cps, hh, nb, 0, HD, last and hh == 1),
                        recb[:])
        ps_cp.release()
        ps_sc.release()
        ps_qk.release()
        p_ae.release()
        p_craw.release()
        p_qkv_in.release()
        p_attn.release()

        # ---------------- proj + residual, LN1 ---------------------------
        p_w2 = tc.alloc_tile_pool(name="p_w2", bufs=1)
        p_x116 = tc.alloc_tile_pool(name="p_x116", bufs=1)
        p_sq = tc.alloc_tile_pool(name="p_sq", bufs=2)
        p_r1 = tc.alloc_tile_pool(name="p_r1", bufs=1)
        ps_ln = tc.alloc_tile_pool(name="ps_ln", bufs=1, space="PSUM")
        ps_bc = tc.alloc_tile_pool(name="ps_bc", bufs=1, space="PSUM")
        ps_pj = tc.alloc_tile_pool(name="ps_pj", bufs=2, space="PSUM")
        w2_sb = p_w2.tile([128, FCH, D], F16)
        nc.sync.dma_start(out=w2_sb[:],
                          in_=wfc2T[:, :].rearrange("(c p) n -> p c n", p=128))
        r1_sb = p_r1.tile([128, DC, N], F16)
        x116_sb = p_x116.tile([128, DC, N], F16)
        x18_sb = (p_x116.tile([128, DC, N], F8, name="x18")
                  if fc1_fp8 else None)

        for et in range(DC):
            ps = ps_pj.tile([128, N], F32, tag="pj", name="pspj")
            for cp in range(0, DC, 2):
                for nb in range(NB):
                    sl = slice(nb * 512, nb * 512 + 512)
                    nc.tensor.matmul(ps[:, sl],
                                     wproj_sb[:, cp:cp + 2,
                                              et * 128:(et + 1) * 128],
                                     ctx_sb[:, cp:cp + 2, sl],
                                     start=(cp == 0), stop=(cp == DC - 2),
                                     perf_mode=DR)
            nc.vector.scalar_tensor_tensor(
                out=r1_sb[:, et, :], in0=ps[:], scalar=s_pj,
                in1=xTb_sb[:, et, :], op0=OP.mult, op1=OP.add)
        ps_pj.release()

        def layer_norm16(src_sb, out_sb, gb_fast, gb_sb, nb, out_sl=None,
                         out8_sb=None, out8_scale=1.0):
            """LN over features for token block nb; src fp16 [128, DC, N]."""
            sl = slice(nb * 512, nb * 512 + 512)
            osl = sl if out_sl is None else out_sl
            s1 = ps_ln.tile([1, 512], F32, tag="s1", name="s1")
            s2 = ps_ln.tile([1, 512], F32, tag="s2", name="s2")
            for c in range(DC):
                nc.tensor.matmul(s1[:], ones16[:], src_sb[:, c, sl],
                                 start=(c == 0), stop=(c == DC - 1))
            for c in range(DC):
                sq = p_sq.tile([128, 512], F16, tag="sq", name="sq")
                nc.vector.tensor_mul(sq[:], src_sb[:, c, sl], src_sb[:, c, sl])
                nc.tensor.matmul(s2[:], ones16[:], sq[:],
                                 start=(c == 0), stop=(c == DC - 1))
            t0 = stats.tile([1, 512], F32, tag="t0", name="t0")
            m2 = stats.tile([1, 512], F32, tag="m2", name="m2")
            var = stats.tile([1, 512], F32, tag="var", name="var")
            lnv = stats.tile([1, 512], F32, tag="lnv", name="lnv")
            a16 = stats.tile([1, 512], F16, tag="a16", name="a16")
            b16 = stats.tile([1, 512], F16, tag="b16", name="b16")
            nc.vector.tensor_scalar_mul(t0[:], in0=s1[:], scalar1=1.0 / D)
            nc.vector.tensor_mul(m2[:], t0[:], t0[:])
            nc.vector.scalar_tensor_tensor(out=var[:], in0=s2[:],
                                           scalar=1.0 / D, in1=m2[:],
                                           op0=OP.mult, op1=OP.subtract)
            # 1/sqrt(var+eps) = exp(-0.5*ln(var+eps)): Ln and Exp share one
            # ACT table, so this costs no table reload next to the softmax
            nc.scalar.activation(out=lnv[:], in_=var[:], func=AF.Ln,
                                 bias=eps_t[:])
            nc.scalar.activation(out=a16[:], in_=lnv[:], func=AF.Exp,
                                 scale=-0.5)
            nc.vector.scalar_tensor_tensor(out=b16[:], in0=a16[:],
                                           scalar=-1.0, in1=t0[:],
                                           op0=OP.mult, op1=OP.mult)
            A_ps = ps_bc.tile([128, 512], F32, tag="A", name="Aps")
            B_ps = ps_bc.tile([128, 512], F32, tag="B", name="Bps")
            nc.tensor.matmul(A_ps[:], ones_row16[:], a16[:],
                             start=True, stop=True)
            nc.tensor.matmul(B_ps[:], ones_row16[:], b16[:],
                             start=True, stop=True)
            A = bc.tile([128, 512], F16, tag="A", name="A")
            B = bc.tile([128, 512], F16, tag="B", name="B")
            nc.scalar.activation(out=A[:], in_=A_ps[:], func=AF.Copy)
            nc.scalar.activation(out=B[:], in_=B_ps[:], func=AF.Copy)
            for c in range(DC):
                u = p_sq.tile([128, 512], F16, tag="u", name="u")
                nc.vector.tensor_mul(u[:], src_sb[:, c, sl], A[:])
                if gb_fast:
                    nc.vector.tensor_add(out_sb[:, c, osl], u[:], B[:])
                else:
                    w = p_sq.tile([128, 512], F16, tag="w", name="w")
                    nc.vector.tensor_add(w[:], u[:], B[:])
                    nc.vector.tensor_scalar(
                        out=out_sb[:, c, osl], in0=w[:],
                        scalar1=gb_sb[:, c:c + 1],
                        scalar2=gb_sb[:, DC + c:DC + c + 1],
                        op0=OP.mult, op1=OP.add)
                if out8_sb is not None:
                    nc.vector.tensor_scalar_mul(out8_sb[:, c, sl],
                                                in0=out_sb[:, c, osl],
                                                scalar1=out8_scale)

        x18_scale = 2.0 ** sc["kx1"] if fc1_fp8 else 1.0
        layer_norm16(r1_sb, x116_sb, sc["gb1_fast"], gb1_sb, 0,
                     out8_sb=x18_sb, out8_scale=x18_scale)
        layer_norm16(r1_sb, x116_sb, sc["gb1_fast"], gb1_sb, 1,
                     out8_sb=x18_sb, out8_scale=x18_scale)
        p_ctx.release()
        p_xTb.release()
        p_r1.release()

        # ---------------- MLP (+ residual), LN2, output ------------------
        p_h = tc.alloc_tile_pool(name="p_h", bufs=1)
        p_y2 = tc.alloc_tile_pool(name="p_y2", bufs=1)
        p_x2 = tc.alloc_tile_pool(name="p_x2", bufs=1)
        h_sb = p_h.tile([128, FCH, N], F16)
        y2_sb = p_y2.tile([128, DC, N], F16)
        x2_sb = p_x2.tile([128, DC, 512], F32)
        ps_f1 = tc.alloc_tile_pool(name="ps_f1", bufs=2, space="PSUM")
        ps_f2 = tc.alloc_tile_pool(name="ps_f2", bufs=2, space="PSUM")

        def fc1(nb):
            sl = slice(nb * 512, nb * 512 + 512)
            for f in range(FCH):
                ps = ps_f1.tile([128, 512], F32, tag="f1", name="psf1")
                if fc1_fp8:
                    for cp in range(0, DC, 2):
                        nc.tensor.matmul(ps[:],
                                         w1_sb[:, cp:cp + 2,
                                               f * 128:(f + 1) * 128],
                                         x18_sb[:, cp:cp + 2, sl],
                                         start=(cp == 0),
                                         stop=(cp == DC - 2), perf_mode=DR)
                else:
                    for c in range(DC):
                        nc.tensor.matmul(ps[:],
                                         w1_sb[:, c, f * 128:(f + 1) * 128],
                                         x116_sb[:, c, sl],
                                         start=(c == 0), stop=(c == DC - 1))
                nc.scalar.activation(out=h_sb[:, f, sl], in_=ps[:],
                                     func=AF.Gelu, scale=s_f1,
                                     bias=bfc1_sb[:, f:f + 1])

        def fc2(nb):
            sl = slice(nb * 512, nb * 512 + 512)
            for et in range(DC):
                ps = ps_f2.tile([128, 512], F32, tag="f2", name="psf2")
                for f in range(FCH):
                    nc.tensor.matmul(ps[:],
                                     w2_sb[:, f, et * 128:(et + 1) * 128],
                                     h_sb[:, f, sl],
                                     start=(f == 0), stop=(f == FCH - 1))
                nc.vector.scalar_tensor_tensor(
                    out=y2_sb[:, et, sl], in0=ps[:],
                    scalar=bfc2_sb[:, et:et + 1], in1=x116_sb[:, et, sl],
                    op0=OP.add, op1=OP.add)

        fc1(0)
        fc1(1)
        fc2(0)
        layer_norm16(y2_sb, x2_sb, sc["gb2_fast"], gb2_sb, 0,
                     out_sl=slice(0, 512))
        for c in range(DC):
            nc.sync.dma_start(out=yT[c * 128:(c + 1) * 128, 0:512],
                              in_=x2_sb[:, c, :])
        fc2(1)
        layer_norm16(y2_sb, x2_sb, sc["gb2_fast"], gb2_sb, 1,
                     out_sl=slice(0, 512))
        for c in range(DC):
            nc.sync.dma_start(out=yT[c * 128:(c + 1) * 128, 512:1024],
                              in_=x2_sb[:, c, :])

        ps_f2.release()
        ps_f1.release()
        ps_pj_ = None
        ps_bc.release()
        ps_ln.release()
        p_x2.release()
        p_y2.release()
        p_h.release()
        p_r1_ = None  # r1 released after LN1
        p_sq.release()
        p_x116.release()
        p_w2.release()
        dscr.release()
        bc.release()
        stats.release()
        p_w1.release()
        const.release()
    return nc


_NC_CACHE = {}


def _get_nc(sc):
    key = tuple(sorted(sc.items()))
    if key not in _NC_CACHE:
        nc = _build(sc)
        _split_excess_waits(nc)
        _NC_CACHE.clear()
        _NC_CACHE[key] = nc
    return _NC_CACHE[key]


def _kexp(amax, target=120.0):
    """power-of-2 scale exponent: amax * 2^k ~= target (<= 240)"""
    return int(np.floor(np.log2(target / max(amax, 1e-30))))


def _q8(a, k):
    return np.clip(a * (2.0 ** k), -240.0, 240.0).astype(NP_F8)


def kernel(x, w_qkv, w_proj, b_proj, w_fc1, b_fc1, w_fc2, b_fc2,
           gamma1, beta1, gamma2, beta2):
    global LAST_RESULT
    x = np.asarray(x, dtype=np.float32)
    w_qkv = np.asarray(w_qkv, dtype=np.float32)
    w_proj = np.asarray(w_proj, dtype=np.float32)
    b_proj = np.asarray(b_proj, dtype=np.float32)
    w_fc1 = np.asarray(w_fc1, dtype=np.float32)
    b_fc1 = np.asarray(b_fc1, dtype=np.float32)
    w_fc2 = np.asarray(w_fc2, dtype=np.float32)
    b_fc2 = np.asarray(b_fc2, dtype=np.float32)
    gamma1 = np.asarray(gamma1, dtype=np.float32)
    beta1 = np.asarray(beta1, dtype=np.float32)
    gamma2 = np.asarray(gamma2, dtype=np.float32)
    beta2 = np.asarray(beta2, dtype=np.float32)

    wq = w_qkv.copy()
    wq[:D] *= HD ** -0.5                     # fold attention scale into Q

    # host-side range probe (fp32, BLAS) to pick exact power-of-2 fp8 scales
    xf = x.reshape(-1, D)
    qh = (xf @ wq[:D].T).reshape(NCORES, N, HEADS, HD)
    kh = (xf @ wq[D:2 * D].T).reshape(NCORES, N, HEADS, HD)
    vh = (xf @ wq[2 * D:].T).reshape(NCORES, N, HEADS, HD)
    smax = 0.0
    cmax = 0.0
    ctxs = np.empty((NCORES, N, HEADS, HD), np.float32)
    for b in range(NCORES):
        for h in range(HEADS):
            s = qh[b, :, h] @ kh[b, :, h].T
            smax = max(smax, float(np.abs(s).max()))
            p = np.exp(s - s.max(axis=-1, keepdims=True))
            cn = (p @ vh[b, :, h]) / p.sum(axis=-1, keepdims=True)
            ctxs[b, :, h] = cn
            cmax = max(cmax, float(np.abs(cn).max()))
    # x1 absmax for the fc1 fp8 input scale
    attn_out = ctxs.reshape(-1, DIM_ := D) @ w_proj.T + b_proj
    r1 = xf + attn_out
    mu = r1.mean(-1, keepdims=True)
    vr = ((r1 - mu) ** 2).mean(-1, keepdims=True)
    x1 = (r1 - mu) / np.sqrt(vr + EPS) * gamma1 + beta1
    x1max = float(np.abs(x1).max())

    sc = {
        "kx": _kexp(np.abs(x).max()),
        "kq": _kexp(np.abs(wq[:D]).max()),
        "kk": _kexp(np.abs(wq[D:2 * D]).max()),
        "kvw": _kexp(np.abs(wq[2 * D:]).max()),
        "kv": _kexp(np.abs(vh).max()),
        "kp": int(np.floor(np.log2(120.0 / np.exp(smax)))),
        "kc": _kexp(cmax),
        "kpr": _kexp(np.abs(w_proj).max()),
        "kx1": _kexp(x1max),
        "kw1": _kexp(np.abs(w_fc1).max()),
        "fc1_fp8": FC1_FP8,
        "gb1_fast": bool(np.all(gamma1 == 1.0) and np.all(beta1 == 0.0)),
        "gb2_fast": bool(np.all(gamma2 == 1.0) and np.all(beta2 == 0.0)),
    }

    wqkv8 = np.concatenate([
        _q8(wq[:D], sc["kq"]), _q8(wq[D:2 * D], sc["kk"]),
        _q8(wq[2 * D:], sc["kvw"])], axis=0)
    wqkvT8 = np.ascontiguousarray(wqkv8.T)
    wprojT8 = np.ascontiguousarray(_q8(w_proj, sc["kpr"]).T)
    if FC1_FP8:
        wfc1T = np.ascontiguousarray(_q8(w_fc1, sc["kw1"]).T)
    else:
        wfc1T = np.ascontiguousarray(w_fc1.T.astype(np.float16))
    wfc2T = np.ascontiguousarray(w_fc2.T.astype(np.float16))

    def cols(v, nchunks):
        return np.ascontiguousarray(v.reshape(nchunks, 128).T)

    shared = {
        "wqkvT8": wqkvT8, "wprojT8": wprojT8,
        "wfc1T": wfc1T, "wfc2T": wfc2T,
        "bfc1C": cols(b_fc1, FCH), "bfc2C": cols(b_fc2, DC),
        "gb1C": np.concatenate([cols(gamma1, DC), cols(beta1, DC)], 1),
        "gb2C": np.concatenate([cols(gamma2, DC), cols(beta2, DC)], 1),
    }
    in_maps = []
    for b in range(NCORES):
        m = dict(shared)
        xt = np.ascontiguousarray(x[b].T)
        m["xTb"] = xt + b_proj[:, None]
        m["xT8"] = _q8(xt, sc["kx"])
        in_maps.append(m)

    nc = _get_nc(sc)
    LAST_RESULT = run_bass_kernel_spmd(nc, in_maps, list(range(NCORES)))
    out = np.stack([np.ascontiguousarray(LAST_RESULT.results[b]["yT"].T)
                    for b in range(NCORES)])
    return out.astype(np.float32)
